# revision 1
# baseline (speedup 1.0000x reference)
"""Trainium2 Bass kernel for nn_Attention3D (GroupNorm + channel-attention + proj + residual).

Sharding: the spatial axis N = d*h*w = 32768 is split across 8 cores (Nc=4096
per core, both batch elements on every core). Two tiny AllReduces:
  AR1: per-channel GroupNorm partial stats (mean, E[x^2])      [128 x 8]  f32
  AR2: channel-attention logits q @ k^T (contracted over N)    [128 x 256] f32

Key algebraic fusions (validated against the reference in numpy):
  - GroupNorm affine is folded into the q/k weight matrix (per-batch row
    scaling) so normalized activations are never materialized.
  - softmax(attn) @ v followed by proj collapses into a single per-batch
    weight G_b = P @ blockdiag(attn) @ Wv (256x256), applied directly to raw
    x, with a per-batch bias vector carrying all bias/affine terms.
  - qkv bias + GroupNorm shift enter the logits as rank-1 corrections added
    after AR2 (exact, from globally-reduced column sums).
"""
import sys

sys.path.insert(0, "/opt/trn_rl_repo")

import numpy as np
import concourse.bass as bass
import concourse.tile as tile
from concourse import mybir
from concourse.bass_utils import run_bass_kernel_spmd

F32 = mybir.dt.float32
F32R = mybir.dt.float32r
ALU = mybir.AluOpType
ACT = mybir.ActivationFunctionType

S = 8            # cores
B, C = 2, 256
N = 32 * 32 * 32
Nc = N // S      # 4096 spatial positions per core
H, HD = 4, 64
G = 8            # groupnorm groups
EPS = 1e-5
SM_SCALE = float(HD) ** -0.5


def _split_excess_waits(nc, max_waits=1):
    """This container's walrus rejects >1 sem wait per instruction; move the
    overflow onto same-engine NoOps inserted immediately before."""
    ctr = 0
    for bb in nc.cur_f.blocks:
        insts = bb.instructions
        i = 0
        while i < len(insts):
            ins = insts[i]
            si = ins.sync_info
            if si is not None and len(si.on_wait) > max_waits:
                waits = list(si.on_wait)
                si.on_wait = waits[:max_waits]
                overflow = waits[max_waits:]
                pos = i
                for j in range(0, len(overflow), max_waits):
                    ctr += 1
                    nop = mybir.InstNoOp(name=f"I-ws-{ctr}", ins=[], outs=[])
                    nop.engine = ins.engine
                    nop.sync_info = mybir.SyncInfo(
                        on_wait=overflow[j : j + max_waits], on_update=[]
                    )
                    insts.insert(pos, nop)
                    pos += 1
                    i += 1
            i += 1


def build_nc(split_waits=True, loop_r=None, upto=99):
    """loop_r=None builds the real kernel. loop_r=R builds a timing variant:
    collectives run once up-front, then the full compute body repeats R times
    inside a hardware For_i loop (for wall-clock slope measurements).
    upto (timing variant only): emit only loop-body phases <= upto:
      0=x reload, 1=stats, 2=post-AR1 prep, 3=pass1, 4=extract+ccdma,
      5=softmax, 6=fused weights, 7=pass2+out."""
    nc = bass.Bass(num_devices=S)

    xs_d = nc.declare_dram_parameter("xs", [2 * B, 128, Nc], F32R, isOutput=False)
    wtqk_d = nc.declare_dram_parameter("wtqk", [C, 512], F32R, isOutput=False)
    wv_d = nc.declare_dram_parameter("wv", [C, C], F32R, isOutput=False)
    pt_d = nc.declare_dram_parameter("pt", [C, C], F32R, isOutput=False)
    gnw_d = nc.declare_dram_parameter("gnw", [C, 1], F32, isOutput=False)
    gnb_d = nc.declare_dram_parameter("gnb", [C, 1], F32, isOutput=False)
    bqk_d = nc.declare_dram_parameter("bqk", [1, 512], F32R, isOutput=False)
    bv_d = nc.declare_dram_parameter("bv", [C, 1], F32R, isOutput=False)
    pb_d = nc.declare_dram_parameter("pb", [1, C], F32, isOutput=False)
    g4_d = nc.declare_dram_parameter("g4", [128, 4], F32, isOutput=False)
    e4_d = nc.declare_dram_parameter("e4", [4, 128], F32, isOutput=False)
    const_d = nc.declare_dram_parameter("konst", [128, 257], F32R, isOutput=False)
    out_d = nc.declare_dram_parameter("out", [2 * B, 128, Nc], F32, isOutput=True)

    cc1i = nc.dram_tensor("cc1i", [128, 8], F32)
    cc1o = nc.dram_tensor("cc1o", [128, 8], F32, addr_space="Shared")
    cc2i = nc.dram_tensor("cc2i", [128, 256], F32)
    cc2o = nc.dram_tensor("cc2o", [128, 256], F32, addr_space="Shared")
    rg = [list(range(S))]

    with tile.TileContext(nc) as tc:
        with (
            tc.tile_pool(name="big", bufs=1) as big,        # resident x / out
            tc.tile_pool(name="wpool", bufs=1) as wpool,    # weights & per-batch mats
            tc.tile_pool(name="small", bufs=1) as small,    # stats / vectors
            tc.tile_pool(name="qkpool", bufs=3) as qkpool,  # pass-1 qk^T staging
            tc.tile_pool(name="p_att", bufs=1, space="PSUM") as p_att,
            tc.tile_pool(name="p_work", bufs=2, space="PSUM") as p_work,
            tc.tile_pool(name="p_misc", bufs=2, space="PSUM") as p_misc,
        ):
            # ---------- phase 0: loads ----------
            x_sb = []  # t = b*2+cb -> [128, Nc]
            for t in range(4):
                xt = big.tile([128, Nc], F32R, tag=f"x{t}", name=f"x{t}")
                nc.sync.dma_start(out=xt[:], in_=xs_d[t])
                x_sb.append(xt)
            wtqk_sb = []
            for k in range(2):
                w = wpool.tile([128, 512], F32R, tag=f"wtqk{k}", name=f"wtqk{k}")
                nc.sync.dma_start(out=w[:], in_=wtqk_d[k * 128:(k + 1) * 128, :])
                wtqk_sb.append(w)
            wv_sb, pt_sb = [], []
            for k in range(2):
                w = wpool.tile([128, C], F32R, tag=f"wv{k}", name=f"wv{k}")
                nc.sync.dma_start(out=w[:], in_=wv_d[k * 128:(k + 1) * 128, :])
                wv_sb.append(w)
                p = wpool.tile([128, C], F32R, tag=f"pt{k}", name=f"pt{k}")
                nc.sync.dma_start(out=p[:], in_=pt_d[k * 128:(k + 1) * 128, :])
                pt_sb.append(p)
            gnw_sb, gnb_sb, bv_sb = [], [], []
            for k in range(2):
                sl = slice(k * 128, (k + 1) * 128)
                gw = small.tile([128, 1], F32, tag=f"gnw{k}", name=f"gnw{k}")
                nc.sync.dma_start(out=gw[:], in_=gnw_d[sl, :])
                gnw_sb.append(gw)
                gb = small.tile([128, 1], F32, tag=f"gnb{k}", name=f"gnb{k}")
                nc.sync.dma_start(out=gb[:], in_=gnb_d[sl, :])
                gnb_sb.append(gb)
                bv = small.tile([128, 1], F32R, tag=f"bv{k}", name=f"bv{k}")
                nc.sync.dma_start(out=bv[:], in_=bv_d[sl, :])
                bv_sb.append(bv)

            pb_sb = small.tile([1, C], F32, tag="pb", name="pb")
            nc.sync.dma_start(out=pb_sb[:], in_=pb_d[:])
            bqk_sb = small.tile([1, 512], F32R, tag="bqk", name="bqk")
            nc.sync.dma_start(out=bqk_sb[:], in_=bqk_d[:])
            g4_sb = small.tile([128, 4], F32, tag="g4", name="g4")
            nc.sync.dma_start(out=g4_sb[:], in_=g4_d[:])
            e4_sb = small.tile([4, 128], F32, tag="e4", name="e4")
            nc.sync.dma_start(out=e4_sb[:], in_=e4_d[:])

            eps41 = small.tile([4, 1], F32, tag="eps", name="eps")
            nc.gpsimd.memset(eps41[:], EPS)
            konst_sb = wpool.tile([128, 257], F32R, tag="konst", name="konst")
            nc.sync.dma_start(out=konst_sb[:], in_=const_d[:])
            one11 = konst_sb[0:1, 256:257]
            scr41 = small.tile([4, 1], F32, tag="scr", name="scr")
            # preload the sqrt activation table while DMAs run
            nc.scalar.activation(out=scr41[:], in_=eps41[:], func=ACT.Sqrt)

            def emit_stats():
                """phase 1: local GroupNorm stats -> st [128, 8] -> cc1i."""
                st = small.tile([128, 8], F32, tag="st", name="st")
                for t in range(4):
                    stats6 = small.tile([128, 8, 6], F32, tag="bn6", name="bn6")
                    for j in range(8):
                        nc.vector.bn_stats(
                            out=stats6[:, j, :], in_=x_sb[t][:, j * 512:(j + 1) * 512]
                        )
                    mv = small.tile([128, 2], F32, tag="mv", name="mv")
                    nc.vector.bn_aggr(out=mv[:], in_=stats6[:])
                    nc.vector.tensor_copy(st[:, t:t + 1], mv[:, 0:1])
                    # E[x^2] = var + mean^2
                    nc.vector.scalar_tensor_tensor(
                        out=st[:, 4 + t:5 + t], in0=mv[:, 0:1], scalar=mv[:, 0:1],
                        in1=mv[:, 1:2], op0=ALU.mult, op1=ALU.add,
                    )
                nc.sync.dma_start(out=cc1i[:], in_=st[:])

            def emit_compute(upto=99):
                """phases 2..7 (generator; yields where AR2 belongs)."""
                st2 = small.tile([128, 8], F32, tag="st2", name="st2")
                nc.sync.dma_start(out=st2[:], in_=cc1o[:])

                # ----- post-AR1 prep -----
                psum_g = p_misc.tile([4, 8], F32, tag="m", name="psum_g")
                nc.tensor.matmul(psum_g[:], g4_sb[:], st2[:], start=True, stop=True)
                gsb = small.tile([4, 8], F32, tag="gsb", name="gsb")
                nc.vector.tensor_copy(gsb[:], psum_g[:])
                var44 = small.tile([4, 4], F32, tag="var44", name="var44")
                nc.vector.scalar_tensor_tensor(
                    out=var44[:], in0=gsb[:, 0:4], scalar=0.0, in1=gsb[:, 0:4],
                    op0=ALU.add, op1=ALU.mult,
                )  # mean^2
                nc.vector.tensor_sub(var44[:], gsb[:, 4:8], var44[:])
                rstd44 = small.tile([4, 4], F32, tag="rstd44", name="rstd44")
                nc.scalar.activation(
                    out=rstd44[:], in_=var44[:], func=ACT.Sqrt, bias=eps41[:], scale=1.0
                )
                nc.vector.reciprocal(out=rstd44[:], in_=rstd44[:])
                # preload the exp table right after the last sqrt
                nc.scalar.activation(out=scr41[:], in_=rstd44[:, 0:1], func=ACT.Exp)

                a_sb = [[None] * 2 for _ in range(B)]
                bb_sb = [[None] * 2 for _ in range(B)]
                wts_sb = [[None] * 2 for _ in range(B)]
                sxg_sb = [[None] * 2 for _ in range(B)]
                for b in range(B):
                    for cb in range(2):
                        t = b * 2 + cb
                        pmean = p_misc.tile([128, 1], F32, tag="m", name="pmean")
                        nc.tensor.matmul(
                            pmean[:], e4_sb[:], gsb[:, t:t + 1], start=True, stop=True
                        )
                        prstd = p_misc.tile([128, 1], F32, tag="m", name="prstd")
                        nc.tensor.matmul(
                            prstd[:], e4_sb[:], rstd44[:, t:t + 1], start=True, stop=True
                        )
                        a = small.tile([128, 1], F32, tag=f"a{t}", name=f"a{t}")
                        nc.vector.tensor_mul(a[:], prstd[:], gnw_sb[cb][:])
                        na = small.tile([128, 1], F32, tag=f"na{t}", name=f"na{t}")
                        nc.scalar.mul(out=na[:], in_=a[:], mul=-1.0)
                        bbv = small.tile([128, 1], F32R, tag=f"bb{t}", name=f"bb{t}")
                        nc.vector.scalar_tensor_tensor(
                            out=bbv[:], in0=pmean[:], scalar=na[:], in1=gnb_sb[cb][:],
                            op0=ALU.mult, op1=ALU.add,
                        )  # gnb - mean*a
                        w = wpool.tile([128, 512], F32R, tag=f"wts{t}", name=f"wts{t}")
                        nc.vector.tensor_scalar_mul(out=w[:], in0=wtqk_sb[cb][:], scalar1=a[:])
                        sx = small.tile([128, 1], F32R, tag=f"sxg{t}", name=f"sxg{t}")
                        nc.scalar.mul(out=sx[:], in_=st2[:, t:t + 1], mul=float(Nc))
                        a_sb[b][cb], bb_sb[b][cb], wts_sb[b][cb], sxg_sb[b][cb] = a, bbv, w, sx

                if upto < 3:
                    return
                # rowbias rb, global colsums Sg, and the rank-1 stacks Lq/Rk
                lq_sb, rk_sb = [], []
                for b in range(B):
                    prb = p_misc.tile([1, 512], F32, tag="m", name="prb")
                    nc.tensor.matmul(prb[:], bb_sb[b][0][:], wtqk_sb[0][:], start=True, stop=False)
                    nc.tensor.matmul(prb[:], bb_sb[b][1][:], wtqk_sb[1][:], start=False, stop=False)
                    nc.tensor.matmul(prb[:], one11, bqk_sb[:], start=False, stop=True)
                    rb = small.tile([1, 512], F32, tag=f"rb{b}", name=f"rb{b}")
                    nc.vector.tensor_copy(rb[:], prb[:])
                    psg = p_misc.tile([1, 512], F32, tag="m", name="psg")
                    nc.tensor.matmul(psg[:], sxg_sb[b][0][:], wts_sb[b][0][:], start=True, stop=False)
                    nc.tensor.matmul(psg[:], sxg_sb[b][1][:], wts_sb[b][1][:], start=False, stop=True)
                    sg = small.tile([1, 512], F32, tag=f"sg{b}", name=f"sg{b}")
                    nc.vector.tensor_copy(sg[:], psg[:])
                    rbn = small.tile([1, 512], F32, tag=f"rbn{b}", name=f"rbn{b}")
                    nc.scalar.mul(out=rbn[:], in_=rb[:], mul=float(N))
                    lq = small.tile([3, 256], F32, tag=f"lq{b}", name=f"lq{b}")
                    nc.sync.dma_start(out=lq[0:1, :], in_=rb[0:1, 0:256])
                    nc.sync.dma_start(out=lq[1:2, :], in_=sg[0:1, 0:256])
                    nc.sync.dma_start(out=lq[2:3, :], in_=rbn[0:1, 0:256])
                    rk = small.tile([3, 256], F32, tag=f"rk{b}", name=f"rk{b}")
                    nc.sync.dma_start(out=rk[0:1, :], in_=sg[0:1, 256:512])
                    nc.sync.dma_start(out=rk[1:2, :], in_=rb[0:1, 256:512])
                    nc.sync.dma_start(out=rk[2:3, :], in_=rb[0:1, 256:512])
                    lq_sb.append(lq)
                    rk_sb.append(rk)

                # ----- pass 1: q/k logits -----
                att_ps = [
                    [
                        p_att.tile([128, 256], F32, tag=f"att{b}{hp}", name=f"att{b}{hp}")
                        for hp in range(2)
                    ]
                    for b in range(B)
                ]
                for b in range(B):
                    for i in range(Nc // 128):
                        nsl = slice(i * 128, (i + 1) * 128)
                        pqk = p_work.tile([128, 512], F32, tag="w", name="pqk")
                        nc.tensor.matmul(
                            pqk[:], x_sb[b * 2][:, nsl], wts_sb[b][0][:], start=True, stop=False
                        )
                        nc.tensor.matmul(
                            pqk[:], x_sb[b * 2 + 1][:, nsl], wts_sb[b][1][:], start=False, stop=True
                        )
                        qkt = qkpool.tile([128, 512], F32R, tag="qkt", name="qkt")
                        if i % 2 == 0:
                            nc.vector.tensor_copy(qkt[:], pqk[:])
                        else:
                            nc.scalar.copy(out=qkt[:], in_=pqk[:])
                        first, last = i == 0, i == Nc // 128 - 1
                        for hp in range(2):
                            nc.tensor.matmul(
                                att_ps[b][hp][:],
                                qkt[:, hp * 128:(hp + 1) * 128],
                                qkt[:, 256:512],
                                start=first, stop=last,
                            )

                if upto < 4:
                    return
                # ----- extract diag blocks -> cc2i -----
                att_all = small.tile([128, 256], F32, tag="att_all", name="att_all")
                for b in range(B):
                    for hp in range(2):
                        t2 = 2 * b + hp
                        csl = slice(t2 * 64, (t2 + 1) * 64)
                        so = hp * 128
                        nc.vector.tensor_copy(att_all[0:64, csl], att_ps[b][hp][0:64, so:so + 64])
                        nc.vector.tensor_copy(att_all[64:128, csl], att_ps[b][hp][64:128, so + 64:so + 128])
                nc.sync.dma_start(out=cc2i[:], in_=att_all[:])
                yield  # AllReduce of cc2i -> cc2o happens here (real kernel)
                attg = small.tile([128, 256], F32, tag="attg", name="attg")
                nc.sync.dma_start(out=attg[:], in_=cc2o[:])

                if upto < 5:
                    return
                # ----- bias corrections + softmax -----
                att_sm = [[None] * 2 for _ in range(B)]
                for b in range(B):
                    for hp in range(2):
                        t2 = 2 * b + hp
                        pc = p_misc.tile([128, 64], F32, tag="m", name="pc")
                        for hh in range(2):
                            h = 2 * hp + hh
                            hsl = slice(h * 64, (h + 1) * 64)
                            nc.tensor.matmul(
                                pc[hh * 64:(hh + 1) * 64, :],
                                lq_sb[b][:, hsl], rk_sb[b][:, hsl],
                                start=True, stop=True, skip_group_check=True,
                            )
                        atc = small.tile([128, 64], F32, tag="atc", name="atc")
                        nc.vector.tensor_add(atc[:], attg[:, t2 * 64:(t2 + 1) * 64], pc[:])
                        negm = small.tile([128, 1], F32, tag="negm", name="negm")
                        nc.vector.reduce_max(
                            out=negm[:], in_=atc[:], axis=mybir.AxisListType.X, negate=True
                        )
                        nc.scalar.mul(out=negm[:], in_=negm[:], mul=SM_SCALE)
                        esb = small.tile([128, 64], F32, tag="esb", name="esb")
                        nc.scalar.activation(
                            out=esb[:], in_=atc[:], func=ACT.Exp,
                            bias=negm[:], scale=SM_SCALE,
                        )
                        ssum = small.tile([128, 1], F32, tag="ssum", name="ssum")
                        nc.vector.reduce_sum(out=ssum[:], in_=esb[:], axis=mybir.AxisListType.X)
                        nc.vector.reciprocal(out=ssum[:], in_=ssum[:])
                        sm = small.tile([128, 64], F32, tag=f"sm{t2}", name=f"sm{t2}")
                        nc.vector.tensor_scalar_mul(out=sm[:], in0=esb[:], scalar1=ssum[:])
                        att_sm[b][hp] = sm

                if upto < 6:
                    return
                # ----- blockdiag + fused per-batch weights -----
                gbt_sb = [[None] * 2 for _ in range(B)]
                mbt_sb = [[None] * 2 for _ in range(B)]
                beta_sb = [[None] * 2 for _ in range(B)]
                for b in range(B):
                    ablk = []
                    for k in range(2):
                        ab = wpool.tile([128, 256], F32R, tag=f"ablk{b}{k}", name=f"ablk{b}{k}")
                        nc.vector.tensor_copy(ab[:], konst_sb[:, 0:256])
                        h0, h1 = 2 * k, 2 * k + 1
                        nc.vector.tensor_copy(ab[0:64, h0 * 64:(h0 + 1) * 64], att_sm[b][k][0:64, :])
                        nc.vector.tensor_copy(ab[64:128, h1 * 64:(h1 + 1) * 64], att_sm[b][k][64:128, :])
                        ablk.append(ab)
                    for m in range(2):
                        pm = p_misc.tile([128, 256], F32, tag="m", name="pm")
                        msl = slice(m * 128, (m + 1) * 128)
                        nc.tensor.matmul(pm[:], ablk[0][:, msl], pt_sb[0][:], start=True, stop=False)
                        nc.tensor.matmul(pm[:], ablk[1][:, msl], pt_sb[1][:], start=False, stop=True)
                        mbt = wpool.tile([128, 256], F32R, tag=f"mbt{b}{m}", name=f"mbt{b}{m}")
                        nc.vector.tensor_copy(mbt[:], pm[:])
                        mbt_sb[b][m] = mbt
                    for g in range(2):
                        pg2 = p_misc.tile([128, 256], F32, tag="m", name="pg2")
                        gsl = slice(g * 128, (g + 1) * 128)
                        nc.tensor.matmul(pg2[:], wv_sb[0][:, gsl], mbt_sb[b][0][:], start=True, stop=False)
                        nc.tensor.matmul(pg2[:], wv_sb[1][:, gsl], mbt_sb[b][1][:], start=False, stop=True)
                        gbt = wpool.tile([128, 256], F32R, tag=f"gbt{b}{g}", name=f"gbt{b}{g}")
                        nc.vector.tensor_copy(gbt[:], pg2[:])
                        gbt_sb[b][g] = gbt
                    pbeta = p_misc.tile([1, C], F32, tag="m", name="pbeta")
                    nc.tensor.matmul(pbeta[:], bb_sb[b][0][:], gbt_sb[b][0][:], start=True, stop=False)
                    nc.tensor.matmul(pbeta[:], bb_sb[b][1][:], gbt_sb[b][1][:], start=False, stop=False)
                    nc.tensor.matmul(pbeta[:], bv_sb[0][:], mbt_sb[b][0][:], start=False, stop=False)
                    nc.tensor.matmul(pbeta[:], bv_sb[1][:], mbt_sb[b][1][:], start=False, stop=True)
                    brow = small.tile([1, C], F32, tag=f"brow{b}", name=f"brow{b}")
                    nc.vector.tensor_add(brow[:], pbeta[:], pb_sb[:])
                    for mo in range(2):
                        bet = small.tile([128, 1], F32, tag=f"beta{b}{mo}", name=f"beta{b}{mo}")
                        nc.sync.dma_start(out=bet[:], in_=brow[0:1, mo * 128:(mo + 1) * 128])
                        beta_sb[b][mo] = bet
                    # fold the GroupNorm scale into G_b (after the bias matmuls read it)
                    for g in range(2):
                        nc.vector.tensor_scalar_mul(
                            out=gbt_sb[b][g][:], in0=gbt_sb[b][g][:], scalar1=a_sb[b][g][:]
                        )

                if upto < 7:
                    return
                # ----- pass 2: out = G_b' x + beta + x -----
                for b in range(B):
                    for mo in range(2):
                        t = b * 2 + mo
                        osb = big.tile([128, Nc], F32, tag=f"o{t}", name=f"o{t}")
                        msl = slice(mo * 128, (mo + 1) * 128)
                        for nt in range(Nc // 512):
                            nsl = slice(nt * 512, (nt + 1) * 512)
                            po = p_work.tile([128, 512], F32, tag="w", name="po")
                            nc.tensor.matmul(po[:], gbt_sb[b][0][:, msl], x_sb[b * 2][:, nsl],
                                             start=True, stop=False)
                            nc.tensor.matmul(po[:], gbt_sb[b][1][:, msl], x_sb[b * 2 + 1][:, nsl],
                                             start=False, stop=True)
                            nc.vector.scalar_tensor_tensor(
                                out=osb[:, nsl], in0=po[:], scalar=beta_sb[b][mo][:],
                                in1=x_sb[t][:, nsl], op0=ALU.add, op1=ALU.add,
                            )
                        nc.sync.dma_start(out=out_d[t], in_=osb[:])

            def ar1():
                nc.gpsimd.collective_compute(
                    "AllReduce", ALU.add, replica_groups=rg, ins=[cc1i[:]], outs=[cc1o[:]]
                )

            def ar2():
                nc.gpsimd.collective_compute(
                    "AllReduce", ALU.add, replica_groups=rg, ins=[cc2i[:]], outs=[cc2o[:]]
                )

            if loop_r is None:
                emit_stats()
                ar1()
                gen = emit_compute()
                next(gen)          # everything up to (and incl.) the cc2i write
                ar2()
                for _ in gen:      # the rest
                    pass
            else:
                # timing variant: collectives once, compute body looped
                emit_stats()
                ar1()
                ar2()
                with tc.For_i(0, loop_r, 1):
                    for t in range(4):
                        nc.sync.dma_start(out=x_sb[t][:], in_=xs_d[t])
                    if upto >= 1:
                        emit_stats()
                    if upto >= 2:
                        for _ in emit_compute(upto):
                            pass

    if split_waits:
        _split_excess_waits(nc)
    return nc


_NC_CACHE = None


def _get_nc():
    global _NC_CACHE
    if _NC_CACHE is None:
        _NC_CACHE = build_nc()
    return _NC_CACHE


def _prep_inputs(x, gn_w, gn_b, qkv_w, qkv_b, proj_w, proj_b):
    x = np.ascontiguousarray(np.asarray(x, np.float32)).reshape(B, C, N)
    qkv_w = np.asarray(qkv_w, np.float32)
    qkv_b = np.asarray(qkv_b, np.float32)
    proj_w = np.asarray(proj_w, np.float32)
    shared = {
        "wtqk": np.ascontiguousarray(qkv_w[0:512].T),
        "wv": np.ascontiguousarray(qkv_w[512:768]),
        "pt": np.ascontiguousarray(proj_w.T),
        "gnw": np.asarray(gn_w, np.float32).reshape(C, 1),
        "gnb": np.asarray(gn_b, np.float32).reshape(C, 1),
        "bqk": qkv_b[0:512].reshape(1, 512),
        "bv": qkv_b[512:768].reshape(C, 1),
        "pb": np.asarray(proj_b, np.float32).reshape(1, C),
    }
    g4 = np.zeros((128, 4), np.float32)
    for p in range(128):
        g4[p, p // 32] = 1.0 / (32.0 * S)
    e4 = np.zeros((4, 128), np.float32)
    for p in range(128):
        e4[p // 32, p] = 1.0
    shared["g4"] = g4
    shared["e4"] = e4
    konst = np.zeros((128, 257), np.float32)
    konst[0, 256] = 1.0
    shared["konst"] = konst
    in_maps = []
    for s in range(S):
        xs = np.ascontiguousarray(x[:, :, s * Nc:(s + 1) * Nc]).reshape(2 * B, 128, Nc)
        in_maps.append({"xs": xs, **{k: v for k, v in shared.items()}})
    return in_maps


def kernel(x, gn_w, gn_b, qkv_w, qkv_b, proj_w, proj_b):
    nc = _get_nc()
    in_maps = _prep_inputs(x, gn_w, gn_b, qkv_w, qkv_b, proj_w, proj_b)
    res = run_bass_kernel_spmd(nc, in_maps, list(range(S)), trace=False)
    shards = [res.results[s]["out"].reshape(B, C, Nc) for s in range(S)]
    return np.concatenate(shards, axis=2).reshape(B, C, 32, 32, 32).astype(np.float32)



# revision 3
# speedup vs baseline: 231.1915x; 231.1915x over previous
"""Trainium2 Bass kernel for nn_Attention3D (GroupNorm + channel-attention + proj + residual).

Sharding: the spatial axis N = d*h*w = 32768 is split across 8 cores (Nc=4096
per core, both batch elements on every core). Two tiny AllReduces:
  AR1: per-channel GroupNorm partial stats (mean, E[x^2])      [128 x 8]  f32
  AR2: channel-attention logits q @ k^T (contracted over N)    [128 x 256] f32

Key algebraic fusions (validated against the reference in numpy):
  - GroupNorm affine is folded into the q/k weight matrix (per-batch row
    scaling) so normalized activations are never materialized.
  - softmax(attn) @ v followed by proj collapses into a single per-batch
    weight G_b = P @ blockdiag(attn) @ Wv (256x256), applied directly to raw
    x, with a per-batch bias vector carrying all bias/affine terms.
  - qkv bias + GroupNorm shift enter the logits as rank-1 corrections added
    after AR2 (exact, from globally-reduced column sums).
"""
import sys

sys.path.insert(0, "/opt/trn_rl_repo")

import numpy as np
import concourse.bass as bass
import concourse.tile as tile
from concourse import mybir
from concourse.bass_utils import run_bass_kernel_spmd

F32 = mybir.dt.float32
F32R = mybir.dt.float32r
ALU = mybir.AluOpType
ACT = mybir.ActivationFunctionType

S = 8            # cores
B, C = 2, 256
N = 32 * 32 * 32
Nc = N // S      # 4096 spatial positions per core
H, HD = 4, 64
G = 8            # groupnorm groups
EPS = 1e-5
SM_SCALE = float(HD) ** -0.5


def _split_excess_waits(nc, max_waits=1):
    """This container's walrus rejects >1 sem wait per instruction; move the
    overflow onto same-engine NoOps inserted immediately before."""
    ctr = 0
    for bb in nc.cur_f.blocks:
        insts = bb.instructions
        i = 0
        while i < len(insts):
            ins = insts[i]
            si = ins.sync_info
            if si is not None and len(si.on_wait) > max_waits:
                waits = list(si.on_wait)
                si.on_wait = waits[:max_waits]
                overflow = waits[max_waits:]
                pos = i
                for j in range(0, len(overflow), max_waits):
                    ctr += 1
                    nop = mybir.InstNoOp(name=f"I-ws-{ctr}", ins=[], outs=[])
                    nop.engine = ins.engine
                    nop.sync_info = mybir.SyncInfo(
                        on_wait=overflow[j : j + max_waits], on_update=[]
                    )
                    insts.insert(pos, nop)
                    pos += 1
                    i += 1
            i += 1


def build_nc(split_waits=True, loop_r=None, upto=99, unroll_r=None):
    """loop_r=None builds the real kernel. loop_r=R builds a timing variant:
    collectives run once up-front, then the full compute body repeats R times
    inside a hardware For_i loop (for wall-clock slope measurements).
    upto (timing variant only): emit only loop-body phases <= upto:
      0=x reload, 1=stats, 2=post-AR1 prep, 3=pass1, 4=extract+ccdma,
      5=softmax, 6=fused weights, 7=pass2+out.
    unroll_r=R (timing variant): the FULL body — input DMAs, stats, AR1,
    compute, AR2, output DMAs — emitted R times sequentially (python
    unroll; collectives inside a HW For_i desync the mesh). Slope between
    two R values = per-invocation HW time including collectives."""
    nc = bass.Bass(num_devices=S)

    xs_d = nc.declare_dram_parameter("xs", [2 * B, 128, Nc], F32R, isOutput=False)
    wtqk_d = nc.declare_dram_parameter("wtqk", [C, 512], F32R, isOutput=False)
    wv_d = nc.declare_dram_parameter("wv", [C, C], F32R, isOutput=False)
    pt_d = nc.declare_dram_parameter("pt", [C, C], F32R, isOutput=False)
    gnw_d = nc.declare_dram_parameter("gnw", [C, 1], F32, isOutput=False)
    gnb_d = nc.declare_dram_parameter("gnb", [C, 1], F32, isOutput=False)
    bqk_d = nc.declare_dram_parameter("bqk", [1, 512], F32R, isOutput=False)
    bv_d = nc.declare_dram_parameter("bv", [C, 1], F32R, isOutput=False)
    pb_d = nc.declare_dram_parameter("pb", [1, C], F32, isOutput=False)
    g4_d = nc.declare_dram_parameter("g4", [128, 4], F32, isOutput=False)
    e4_d = nc.declare_dram_parameter("e4", [4, 128], F32, isOutput=False)
    const_d = nc.declare_dram_parameter("konst", [128, 257], F32R, isOutput=False)
    out_d = nc.declare_dram_parameter("out", [2 * B, 128, Nc], F32, isOutput=True)

    cc1i = nc.dram_tensor("cc1i", [128, 8], F32)
    cc1o = nc.dram_tensor("cc1o", [128, 8], F32, addr_space="Shared")
    cc2i = nc.dram_tensor("cc2i", [128, 256], F32)
    cc2o = nc.dram_tensor("cc2o", [128, 256], F32, addr_space="Shared")
    rg = [list(range(S))]

    with tile.TileContext(nc) as tc:
        with (
            tc.tile_pool(name="big", bufs=1) as big,        # resident x / out
            tc.tile_pool(name="wpool", bufs=1) as wpool,    # weights & per-batch mats
            tc.tile_pool(name="small", bufs=1) as small,    # stats / vectors
            tc.tile_pool(name="qkpool", bufs=3) as qkpool,  # pass-1 qk^T staging
            tc.tile_pool(name="p_att", bufs=1, space="PSUM") as p_att,
            tc.tile_pool(name="p_work", bufs=2, space="PSUM") as p_work,
            tc.tile_pool(name="p_misc", bufs=2, space="PSUM") as p_misc,
        ):
            # ---------- phase 0: loads ----------
            x_sb = []  # t = b*2+cb -> [128, Nc]
            for t in range(4):
                xt = big.tile([128, Nc], F32R, tag=f"x{t}", name=f"x{t}")
                nc.sync.dma_start(out=xt[:], in_=xs_d[t])
                x_sb.append(xt)
            wtqk_sb = []
            for k in range(2):
                w = wpool.tile([128, 512], F32R, tag=f"wtqk{k}", name=f"wtqk{k}")
                nc.sync.dma_start(out=w[:], in_=wtqk_d[k * 128:(k + 1) * 128, :])
                wtqk_sb.append(w)
            wv_sb, pt_sb = [], []
            for k in range(2):
                w = wpool.tile([128, C], F32R, tag=f"wv{k}", name=f"wv{k}")
                nc.sync.dma_start(out=w[:], in_=wv_d[k * 128:(k + 1) * 128, :])
                wv_sb.append(w)
                p = wpool.tile([128, C], F32R, tag=f"pt{k}", name=f"pt{k}")
                nc.sync.dma_start(out=p[:], in_=pt_d[k * 128:(k + 1) * 128, :])
                pt_sb.append(p)
            gnw_sb, gnb_sb, bv_sb = [], [], []
            for k in range(2):
                sl = slice(k * 128, (k + 1) * 128)
                gw = small.tile([128, 1], F32, tag=f"gnw{k}", name=f"gnw{k}")
                nc.sync.dma_start(out=gw[:], in_=gnw_d[sl, :])
                gnw_sb.append(gw)
                gb = small.tile([128, 1], F32, tag=f"gnb{k}", name=f"gnb{k}")
                nc.sync.dma_start(out=gb[:], in_=gnb_d[sl, :])
                gnb_sb.append(gb)
                bv = small.tile([128, 1], F32R, tag=f"bv{k}", name=f"bv{k}")
                nc.sync.dma_start(out=bv[:], in_=bv_d[sl, :])
                bv_sb.append(bv)

            pb_sb = small.tile([1, C], F32, tag="pb", name="pb")
            nc.sync.dma_start(out=pb_sb[:], in_=pb_d[:])
            bqk_sb = small.tile([1, 512], F32R, tag="bqk", name="bqk")
            nc.sync.dma_start(out=bqk_sb[:], in_=bqk_d[:])
            g4_sb = small.tile([128, 4], F32, tag="g4", name="g4")
            nc.sync.dma_start(out=g4_sb[:], in_=g4_d[:])
            e4_sb = small.tile([4, 128], F32, tag="e4", name="e4")
            nc.sync.dma_start(out=e4_sb[:], in_=e4_d[:])

            eps41 = small.tile([4, 1], F32, tag="eps", name="eps")
            nc.gpsimd.memset(eps41[:], EPS)
            konst_sb = wpool.tile([128, 257], F32R, tag="konst", name="konst")
            nc.sync.dma_start(out=konst_sb[:], in_=const_d[:])
            one11 = konst_sb[0:1, 256:257]
            scr41 = small.tile([4, 1], F32, tag="scr", name="scr")
            # preload the sqrt activation table while DMAs run
            nc.scalar.activation(out=scr41[:], in_=eps41[:], func=ACT.Sqrt)

            def emit_stats():
                """phase 1: local GroupNorm stats -> st [128, 8] -> cc1i."""
                st = small.tile([128, 8], F32, tag="st", name="st")
                for t in range(4):
                    stats6 = small.tile([128, 8, 6], F32, tag="bn6", name="bn6")
                    for j in range(8):
                        nc.vector.bn_stats(
                            out=stats6[:, j, :], in_=x_sb[t][:, j * 512:(j + 1) * 512]
                        )
                    mv = small.tile([128, 2], F32, tag="mv", name="mv")
                    nc.vector.bn_aggr(out=mv[:], in_=stats6[:])
                    nc.vector.tensor_copy(st[:, t:t + 1], mv[:, 0:1])
                    # E[x^2] = var + mean^2
                    nc.vector.scalar_tensor_tensor(
                        out=st[:, 4 + t:5 + t], in0=mv[:, 0:1], scalar=mv[:, 0:1],
                        in1=mv[:, 1:2], op0=ALU.mult, op1=ALU.add,
                    )
                nc.sync.dma_start(out=cc1i[:], in_=st[:])

            def emit_compute(upto=99):
                """phases 2..7 (generator; yields where AR2 belongs)."""
                st2 = small.tile([128, 8], F32, tag="st2", name="st2")
                nc.sync.dma_start(out=st2[:], in_=cc1o[:])

                # ----- post-AR1 prep -----
                psum_g = p_misc.tile([4, 8], F32, tag="m", name="psum_g")
                nc.tensor.matmul(psum_g[:], g4_sb[:], st2[:], start=True, stop=True)
                gsb = small.tile([4, 8], F32, tag="gsb", name="gsb")
                nc.vector.tensor_copy(gsb[:], psum_g[:])
                var44 = small.tile([4, 4], F32, tag="var44", name="var44")
                nc.vector.scalar_tensor_tensor(
                    out=var44[:], in0=gsb[:, 0:4], scalar=0.0, in1=gsb[:, 0:4],
                    op0=ALU.add, op1=ALU.mult,
                )  # mean^2
                nc.vector.tensor_sub(var44[:], gsb[:, 4:8], var44[:])
                rstd44 = small.tile([4, 4], F32, tag="rstd44", name="rstd44")
                nc.scalar.activation(
                    out=rstd44[:], in_=var44[:], func=ACT.Sqrt, bias=eps41[:], scale=1.0
                )
                nc.vector.reciprocal(out=rstd44[:], in_=rstd44[:])
                # preload the exp table right after the last sqrt
                nc.scalar.activation(out=scr41[:], in_=rstd44[:, 0:1], func=ACT.Exp)

                a_sb = [[None] * 2 for _ in range(B)]
                bb_sb = [[None] * 2 for _ in range(B)]
                wts_sb = [[None] * 2 for _ in range(B)]
                sxg_sb = [[None] * 2 for _ in range(B)]
                for b in range(B):
                    for cb in range(2):
                        t = b * 2 + cb
                        pmean = p_misc.tile([128, 1], F32, tag="m", name="pmean")
                        nc.tensor.matmul(
                            pmean[:], e4_sb[:], gsb[:, t:t + 1], start=True, stop=True
                        )
                        prstd = p_misc.tile([128, 1], F32, tag="m", name="prstd")
                        nc.tensor.matmul(
                            prstd[:], e4_sb[:], rstd44[:, t:t + 1], start=True, stop=True
                        )
                        a = small.tile([128, 1], F32, tag=f"a{t}", name=f"a{t}")
                        nc.vector.tensor_mul(a[:], prstd[:], gnw_sb[cb][:])
                        na = small.tile([128, 1], F32, tag=f"na{t}", name=f"na{t}")
                        nc.scalar.mul(out=na[:], in_=a[:], mul=-1.0)
                        bbv = small.tile([128, 1], F32R, tag=f"bb{t}", name=f"bb{t}")
                        nc.vector.scalar_tensor_tensor(
                            out=bbv[:], in0=pmean[:], scalar=na[:], in1=gnb_sb[cb][:],
                            op0=ALU.mult, op1=ALU.add,
                        )  # gnb - mean*a
                        w = wpool.tile([128, 512], F32R, tag=f"wts{t}", name=f"wts{t}")
                        nc.vector.tensor_scalar_mul(out=w[:], in0=wtqk_sb[cb][:], scalar1=a[:])
                        sx = small.tile([128, 1], F32R, tag=f"sxg{t}", name=f"sxg{t}")
                        nc.scalar.mul(out=sx[:], in_=st2[:, t:t + 1], mul=float(Nc))
                        a_sb[b][cb], bb_sb[b][cb], wts_sb[b][cb], sxg_sb[b][cb] = a, bbv, w, sx

                if upto < 3:
                    return
                # rowbias rb, global colsums Sg, and the rank-1 stacks Lq/Rk
                lq_sb, rk_sb = [], []
                for b in range(B):
                    prb = p_misc.tile([1, 512], F32, tag="m", name="prb")
                    nc.tensor.matmul(prb[:], bb_sb[b][0][:], wtqk_sb[0][:], start=True, stop=False)
                    nc.tensor.matmul(prb[:], bb_sb[b][1][:], wtqk_sb[1][:], start=False, stop=False)
                    nc.tensor.matmul(prb[:], one11, bqk_sb[:], start=False, stop=True)
                    rb = small.tile([1, 512], F32, tag=f"rb{b}", name=f"rb{b}")
                    nc.vector.tensor_copy(rb[:], prb[:])
                    psg = p_misc.tile([1, 512], F32, tag="m", name="psg")
                    nc.tensor.matmul(psg[:], sxg_sb[b][0][:], wts_sb[b][0][:], start=True, stop=False)
                    nc.tensor.matmul(psg[:], sxg_sb[b][1][:], wts_sb[b][1][:], start=False, stop=True)
                    sg = small.tile([1, 512], F32, tag=f"sg{b}", name=f"sg{b}")
                    nc.vector.tensor_copy(sg[:], psg[:])
                    rbn = small.tile([1, 512], F32, tag=f"rbn{b}", name=f"rbn{b}")
                    nc.scalar.mul(out=rbn[:], in_=rb[:], mul=float(N))
                    lq = small.tile([3, 256], F32, tag=f"lq{b}", name=f"lq{b}")
                    nc.sync.dma_start(out=lq[0:1, :], in_=rb[0:1, 0:256])
                    nc.sync.dma_start(out=lq[1:2, :], in_=sg[0:1, 0:256])
                    nc.sync.dma_start(out=lq[2:3, :], in_=rbn[0:1, 0:256])
                    rk = small.tile([3, 256], F32, tag=f"rk{b}", name=f"rk{b}")
                    nc.sync.dma_start(out=rk[0:1, :], in_=sg[0:1, 256:512])
                    nc.sync.dma_start(out=rk[1:2, :], in_=rb[0:1, 256:512])
                    nc.sync.dma_start(out=rk[2:3, :], in_=rb[0:1, 256:512])
                    lq_sb.append(lq)
                    rk_sb.append(rk)

                # ----- pass 1: q/k logits -----
                att_ps = [
                    [
                        p_att.tile([128, 256], F32, tag=f"att{b}{hp}", name=f"att{b}{hp}")
                        for hp in range(2)
                    ]
                    for b in range(B)
                ]
                for b in range(B):
                    for i in range(Nc // 128):
                        nsl = slice(i * 128, (i + 1) * 128)
                        pqk = p_work.tile([128, 512], F32, tag="w", name="pqk")
                        nc.tensor.matmul(
                            pqk[:], x_sb[b * 2][:, nsl], wts_sb[b][0][:], start=True, stop=False
                        )
                        nc.tensor.matmul(
                            pqk[:], x_sb[b * 2 + 1][:, nsl], wts_sb[b][1][:], start=False, stop=True
                        )
                        qkt = qkpool.tile([128, 512], F32R, tag="qkt", name="qkt")
                        if i % 2 == 0:
                            nc.vector.tensor_copy(qkt[:], pqk[:])
                        else:
                            nc.scalar.copy(out=qkt[:], in_=pqk[:])
                        first, last = i == 0, i == Nc // 128 - 1
                        for hp in range(2):
                            nc.tensor.matmul(
                                att_ps[b][hp][:],
                                qkt[:, hp * 128:(hp + 1) * 128],
                                qkt[:, 256:512],
                                start=first, stop=last,
                            )

                if upto < 4:
                    return
                # ----- extract diag blocks -> cc2i -----
                att_all = small.tile([128, 256], F32, tag="att_all", name="att_all")
                for b in range(B):
                    for hp in range(2):
                        t2 = 2 * b + hp
                        csl = slice(t2 * 64, (t2 + 1) * 64)
                        so = hp * 128
                        nc.vector.tensor_copy(att_all[0:64, csl], att_ps[b][hp][0:64, so:so + 64])
                        nc.vector.tensor_copy(att_all[64:128, csl], att_ps[b][hp][64:128, so + 64:so + 128])
                nc.sync.dma_start(out=cc2i[:], in_=att_all[:])
                yield  # AllReduce of cc2i -> cc2o happens here (real kernel)
                attg = small.tile([128, 256], F32, tag="attg", name="attg")
                nc.sync.dma_start(out=attg[:], in_=cc2o[:])

                if upto < 5:
                    return
                # ----- bias corrections + softmax -----
                att_sm = [[None] * 2 for _ in range(B)]
                for b in range(B):
                    for hp in range(2):
                        t2 = 2 * b + hp
                        pc = p_misc.tile([128, 64], F32, tag="m", name="pc")
                        for hh in range(2):
                            h = 2 * hp + hh
                            hsl = slice(h * 64, (h + 1) * 64)
                            nc.tensor.matmul(
                                pc[hh * 64:(hh + 1) * 64, :],
                                lq_sb[b][:, hsl], rk_sb[b][:, hsl],
                                start=True, stop=True, skip_group_check=True,
                            )
                        atc = small.tile([128, 64], F32, tag="atc", name="atc")
                        nc.vector.tensor_add(atc[:], attg[:, t2 * 64:(t2 + 1) * 64], pc[:])
                        negm = small.tile([128, 1], F32, tag="negm", name="negm")
                        nc.vector.reduce_max(
                            out=negm[:], in_=atc[:], axis=mybir.AxisListType.X, negate=True
                        )
                        nc.scalar.mul(out=negm[:], in_=negm[:], mul=SM_SCALE)
                        esb = small.tile([128, 64], F32, tag="esb", name="esb")
                        nc.scalar.activation(
                            out=esb[:], in_=atc[:], func=ACT.Exp,
                            bias=negm[:], scale=SM_SCALE,
                        )
                        ssum = small.tile([128, 1], F32, tag="ssum", name="ssum")
                        nc.vector.reduce_sum(out=ssum[:], in_=esb[:], axis=mybir.AxisListType.X)
                        nc.vector.reciprocal(out=ssum[:], in_=ssum[:])
                        sm = small.tile([128, 64], F32, tag=f"sm{t2}", name=f"sm{t2}")
                        nc.vector.tensor_scalar_mul(out=sm[:], in0=esb[:], scalar1=ssum[:])
                        att_sm[b][hp] = sm

                if upto < 6:
                    return
                # ----- blockdiag + fused per-batch weights -----
                gbt_sb = [[None] * 2 for _ in range(B)]
                mbt_sb = [[None] * 2 for _ in range(B)]
                beta_sb = [[None] * 2 for _ in range(B)]
                for b in range(B):
                    ablk = []
                    for k in range(2):
                        ab = wpool.tile([128, 256], F32R, tag=f"ablk{b}{k}", name=f"ablk{b}{k}")
                        nc.vector.tensor_copy(ab[:], konst_sb[:, 0:256])
                        h0, h1 = 2 * k, 2 * k + 1
                        nc.vector.tensor_copy(ab[0:64, h0 * 64:(h0 + 1) * 64], att_sm[b][k][0:64, :])
                        nc.vector.tensor_copy(ab[64:128, h1 * 64:(h1 + 1) * 64], att_sm[b][k][64:128, :])
                        ablk.append(ab)
                    for m in range(2):
                        pm = p_misc.tile([128, 256], F32, tag="m", name="pm")
                        msl = slice(m * 128, (m + 1) * 128)
                        nc.tensor.matmul(pm[:], ablk[0][:, msl], pt_sb[0][:], start=True, stop=False)
                        nc.tensor.matmul(pm[:], ablk[1][:, msl], pt_sb[1][:], start=False, stop=True)
                        mbt = wpool.tile([128, 256], F32R, tag=f"mbt{b}{m}", name=f"mbt{b}{m}")
                        nc.vector.tensor_copy(mbt[:], pm[:])
                        mbt_sb[b][m] = mbt
                    for g in range(2):
                        pg2 = p_misc.tile([128, 256], F32, tag="m", name="pg2")
                        gsl = slice(g * 128, (g + 1) * 128)
                        nc.tensor.matmul(pg2[:], wv_sb[0][:, gsl], mbt_sb[b][0][:], start=True, stop=False)
                        nc.tensor.matmul(pg2[:], wv_sb[1][:, gsl], mbt_sb[b][1][:], start=False, stop=True)
                        gbt = wpool.tile([128, 256], F32R, tag=f"gbt{b}{g}", name=f"gbt{b}{g}")
                        nc.vector.tensor_copy(gbt[:], pg2[:])
                        gbt_sb[b][g] = gbt
                    pbeta = p_misc.tile([1, C], F32, tag="m", name="pbeta")
                    nc.tensor.matmul(pbeta[:], bb_sb[b][0][:], gbt_sb[b][0][:], start=True, stop=False)
                    nc.tensor.matmul(pbeta[:], bb_sb[b][1][:], gbt_sb[b][1][:], start=False, stop=False)
                    nc.tensor.matmul(pbeta[:], bv_sb[0][:], mbt_sb[b][0][:], start=False, stop=False)
                    nc.tensor.matmul(pbeta[:], bv_sb[1][:], mbt_sb[b][1][:], start=False, stop=True)
                    brow = small.tile([1, C], F32, tag=f"brow{b}", name=f"brow{b}")
                    nc.vector.tensor_add(brow[:], pbeta[:], pb_sb[:])
                    for mo in range(2):
                        bet = small.tile([128, 1], F32, tag=f"beta{b}{mo}", name=f"beta{b}{mo}")
                        nc.sync.dma_start(out=bet[:], in_=brow[0:1, mo * 128:(mo + 1) * 128])
                        beta_sb[b][mo] = bet
                    # fold the GroupNorm scale into G_b (after the bias matmuls read it)
                    for g in range(2):
                        nc.vector.tensor_scalar_mul(
                            out=gbt_sb[b][g][:], in0=gbt_sb[b][g][:], scalar1=a_sb[b][g][:]
                        )

                if upto < 7:
                    return
                # ----- pass 2: out = G_b' x + beta + x -----
                for b in range(B):
                    for mo in range(2):
                        t = b * 2 + mo
                        osb = big.tile([128, Nc], F32, tag=f"o{t}", name=f"o{t}")
                        msl = slice(mo * 128, (mo + 1) * 128)
                        for nt in range(Nc // 512):
                            nsl = slice(nt * 512, (nt + 1) * 512)
                            po = p_work.tile([128, 512], F32, tag="w", name="po")
                            nc.tensor.matmul(po[:], gbt_sb[b][0][:, msl], x_sb[b * 2][:, nsl],
                                             start=True, stop=False)
                            nc.tensor.matmul(po[:], gbt_sb[b][1][:, msl], x_sb[b * 2 + 1][:, nsl],
                                             start=False, stop=True)
                            nc.vector.scalar_tensor_tensor(
                                out=osb[:, nsl], in0=po[:], scalar=beta_sb[b][mo][:],
                                in1=x_sb[t][:, nsl], op0=ALU.add, op1=ALU.add,
                            )
                        nc.sync.dma_start(out=out_d[t], in_=osb[:])

            def ar1():
                nc.gpsimd.collective_compute(
                    "AllReduce", ALU.add, replica_groups=rg, ins=[cc1i[:]], outs=[cc1o[:]]
                )

            def ar2():
                nc.gpsimd.collective_compute(
                    "AllReduce", ALU.add, replica_groups=rg, ins=[cc2i[:]], outs=[cc2o[:]]
                )

            if loop_r is None:
                for r in range(unroll_r or 1):
                    if r > 0:
                        # re-issue every input DMA so each unrolled body pays
                        # the same HBM traffic as a fresh invocation
                        for t in range(4):
                            nc.sync.dma_start(out=x_sb[t][:], in_=xs_d[t])
                        for k in range(2):
                            nc.sync.dma_start(out=wtqk_sb[k][:], in_=wtqk_d[k * 128:(k + 1) * 128, :])
                            nc.sync.dma_start(out=wv_sb[k][:], in_=wv_d[k * 128:(k + 1) * 128, :])
                            nc.sync.dma_start(out=pt_sb[k][:], in_=pt_d[k * 128:(k + 1) * 128, :])
                    emit_stats()
                    ar1()
                    gen = emit_compute()
                    next(gen)      # everything up to (and incl.) the cc2i write
                    ar2()
                    for _ in gen:  # the rest
                        pass
            else:
                # timing variant: collectives once, compute body looped
                emit_stats()
                ar1()
                ar2()
                with tc.For_i(0, loop_r, 1):
                    for t in range(4):
                        nc.sync.dma_start(out=x_sb[t][:], in_=xs_d[t])
                    if upto >= 1:
                        emit_stats()
                    if upto >= 2:
                        for _ in emit_compute(upto):
                            pass

    if split_waits:
        _split_excess_waits(nc)
    return nc


_NC_CACHE = None


def _get_nc():
    global _NC_CACHE
    if _NC_CACHE is None:
        _NC_CACHE = build_nc()
    return _NC_CACHE


def _prep_inputs(x, gn_w, gn_b, qkv_w, qkv_b, proj_w, proj_b):
    x = np.ascontiguousarray(np.asarray(x, np.float32)).reshape(B, C, N)
    qkv_w = np.asarray(qkv_w, np.float32)
    qkv_b = np.asarray(qkv_b, np.float32)
    proj_w = np.asarray(proj_w, np.float32)
    shared = {
        "wtqk": np.ascontiguousarray(qkv_w[0:512].T),
        "wv": np.ascontiguousarray(qkv_w[512:768]),
        "pt": np.ascontiguousarray(proj_w.T),
        "gnw": np.asarray(gn_w, np.float32).reshape(C, 1),
        "gnb": np.asarray(gn_b, np.float32).reshape(C, 1),
        "bqk": qkv_b[0:512].reshape(1, 512),
        "bv": qkv_b[512:768].reshape(C, 1),
        "pb": np.asarray(proj_b, np.float32).reshape(1, C),
    }
    g4 = np.zeros((128, 4), np.float32)
    for p in range(128):
        g4[p, p // 32] = 1.0 / (32.0 * S)
    e4 = np.zeros((4, 128), np.float32)
    for p in range(128):
        e4[p // 32, p] = 1.0
    shared["g4"] = g4
    shared["e4"] = e4
    konst = np.zeros((128, 257), np.float32)
    konst[0, 256] = 1.0
    shared["konst"] = konst
    in_maps = []
    for s in range(S):
        xs = np.ascontiguousarray(x[:, :, s * Nc:(s + 1) * Nc]).reshape(2 * B, 128, Nc)
        in_maps.append({"xs": xs, **{k: v for k, v in shared.items()}})
    return in_maps


def kernel(x, gn_w, gn_b, qkv_w, qkv_b, proj_w, proj_b):
    nc = _get_nc()
    in_maps = _prep_inputs(x, gn_w, gn_b, qkv_w, qkv_b, proj_w, proj_b)
    res = run_bass_kernel_spmd(nc, in_maps, list(range(S)), trace=False)
    shards = [res.results[s]["out"].reshape(B, C, Nc) for s in range(S)]
    return np.concatenate(shards, axis=2).reshape(B, C, 32, 32, 32).astype(np.float32)



# revision 10
# speedup vs baseline: 898.6129x; 3.8869x over previous
"""Trainium2 Bass kernel for nn_Attention3D (GroupNorm + channel-attention + proj + residual).

Sharding: the spatial axis N = d*h*w = 32768 is split across 8 cores (Nc=4096
per core, both batch elements on every core). ONE AllReduce of [128, 1028]:
per batch and channel-half, a [128, 257] block [G | S] where G = X_s X_s^T
(Gram, contracted over the core's spatial positions) and S = X_s 1.

Key algebra (validated against the reference in numpy):
  - Channel-attention logits contract over N, so
        L_b = A G_b B^T + (A S) w^T + u (B S)^T + N u w^T
    with A = Wq diag(alpha), B = Wk diag(alpha), u = Wq beta + bq,
    w = Wk beta + bk. Only [G | S] needs the network; q/k are never
    materialized, which deletes the 47us q/k projection pass.
  - GroupNorm mean/var derive from the SAME payload (mean from S, E[x^2]
    from diag G), so nothing upstream of the AllReduce touches the [c,n]
    copy of x -- the Gram front half and the post-AR back half decouple,
    and unrolled bodies software-pipeline (body r's Gram + AllReduce are
    emitted before body r-1's post-AR compute, hiding AR latency under
    PE work with no deadlock).
  - softmax(attn) @ v followed by proj collapses into a per-batch weight
    G_b' = P blockdiag(attn) (Wv diag(alpha)) applied directly to raw x,
    with a per-batch bias vector carrying all bias/affine terms.
  - x is staged in BOTH layouts ([c,n] for pass 2 + residual, [n,c] tiles
    with a baked ones-column for the Gram pass); the host transpose is
    free. Pass 2 adds bias+residual in-place in PSUM and DMAs straight
    to DRAM, so no output SBUF staging is needed.
"""
import sys

sys.path.insert(0, "/opt/trn_rl_repo")

import numpy as np
import concourse.bass as bass
import concourse.tile as tile
from concourse import mybir
from concourse.bass_utils import run_bass_kernel_spmd

F32 = mybir.dt.float32
F32R = mybir.dt.float32r
ALU = mybir.AluOpType
ACT = mybir.ActivationFunctionType

S = 8            # cores
B, C = 2, 256
N = 32 * 32 * 32
Nc = N // S      # 4096 spatial positions per core
H, HD = 4, 64
G = 8            # groupnorm groups
EPS = 1e-5
SM_SCALE = float(HD) ** -0.5
NT = Nc // 128   # 32 [n,c] tiles per batch
TW = C + 2       # tile width incl. ones column (+zero pad: fp32r needs even moving dim)
CCW = 4 * TW     # AllReduce payload width


def _split_excess_waits(nc, max_waits=1):
    """This container's walrus rejects >1 sem wait per instruction; move the
    overflow onto same-engine NoOps inserted immediately before."""
    ctr = 0
    for bb in nc.cur_f.blocks:
        insts = bb.instructions
        i = 0
        while i < len(insts):
            ins = insts[i]
            si = ins.sync_info
            if si is not None and len(si.on_wait) > max_waits:
                waits = list(si.on_wait)
                si.on_wait = waits[:max_waits]
                overflow = waits[max_waits:]
                pos = i
                for j in range(0, len(overflow), max_waits):
                    ctr += 1
                    nop = mybir.InstNoOp(name=f"I-ws-{ctr}", ins=[], outs=[])
                    nop.engine = ins.engine
                    nop.sync_info = mybir.SyncInfo(
                        on_wait=overflow[j : j + max_waits], on_update=[]
                    )
                    insts.insert(pos, nop)
                    pos += 1
                    i += 1
            i += 1


def build_nc(split_waits=True, loop_r=None, upto=99, unroll_r=None):
    """loop_r=None builds the real kernel. loop_r=R builds a timing variant:
    the collective runs once up-front, then the compute body repeats R times
    inside a hardware For_i loop. upto (timing variant only): emit only
    loop-body phases <= upto: 0=x reload, 2=gram+ccdma, 3=post-AR prep,
    4=logits, 5=softmax, 6=fused weights, 7=pass2+out.
    unroll_r=R: the FULL body (input DMAs, Gram, AllReduce, logits, softmax,
    pass 2, output DMAs) emitted R times, software-pipelined (collectives
    inside a HW For_i desync the mesh). Slope between two R values =
    per-invocation HW time including the collective."""
    nc = bass.Bass(num_devices=S)

    xs_d = nc.declare_dram_parameter("xs", [2 * B, 128, Nc], F32R, isOutput=False)
    xt_d = nc.declare_dram_parameter("xt", [B, 128, NT * TW], F32R, isOutput=False)
    wtqk_d = nc.declare_dram_parameter("wtqk", [C, 512], F32R, isOutput=False)
    wv_d = nc.declare_dram_parameter("wv", [C, C], F32R, isOutput=False)
    pt_d = nc.declare_dram_parameter("pt", [C, C], F32R, isOutput=False)
    gnw_d = nc.declare_dram_parameter("gnw", [C, 1], F32, isOutput=False)
    gnb_d = nc.declare_dram_parameter("gnb", [C, 1], F32, isOutput=False)
    bqk_d = nc.declare_dram_parameter("bqk", [1, 512], F32R, isOutput=False)
    bv_d = nc.declare_dram_parameter("bv", [C, 1], F32R, isOutput=False)
    pb_d = nc.declare_dram_parameter("pb", [1, C], F32, isOutput=False)
    g4_d = nc.declare_dram_parameter("g4", [128, 4], F32, isOutput=False)
    e4_d = nc.declare_dram_parameter("e4", [4, 128], F32, isOutput=False)
    const_d = nc.declare_dram_parameter("konst", [128, 257], F32R, isOutput=False)
    dmask_d = nc.declare_dram_parameter("dmask", [128, 512], F32, isOutput=False)
    out_d = nc.declare_dram_parameter("out", [2 * B, 128, Nc], F32, isOutput=True)

    cci = nc.dram_tensor("cci", [128, CCW], F32R)
    cco = nc.dram_tensor("cco", [128, CCW], F32R, addr_space="Shared")
    rg = [list(range(S))]

    with tile.TileContext(nc) as tc:
        with (
            tc.tile_pool(name="big", bufs=1) as big,        # resident x (both layouts)
            tc.tile_pool(name="wpool", bufs=1) as wpool,    # weights & per-batch mats
            tc.tile_pool(name="small", bufs=1) as small,    # stats / vectors
            tc.tile_pool(name="ochunk", bufs=4) as ochunk,  # pass-2 output staging
            tc.tile_pool(name="p_g", bufs=1, space="PSUM") as p_g,
            tc.tile_pool(name="p_work", bufs=2, space="PSUM") as p_work,
            tc.tile_pool(name="p_misc", bufs=2, space="PSUM") as p_misc,
        ):
            # ---------- one-time loads ----------
            x_sb = []  # t = b*2+cb -> [128, Nc] channel-major
            for t in range(4):
                xt_ = big.tile([128, Nc], F32R, tag=f"x{t}", name=f"x{t}")
                nc.sync.dma_start(out=xt_[:], in_=xs_d[t])
                x_sb.append(xt_)
            xt_sb = []  # b -> [128, NT*TW] spatial-major tiles (+ones col)
            for b in range(B):
                xt_ = big.tile([128, NT * TW], F32R, tag=f"xt{b}", name=f"xt{b}")
                nc.sync.dma_start(out=xt_[:], in_=xt_d[b])
                xt_sb.append(xt_)
            wtqk_sb = []
            for k in range(2):
                w = wpool.tile([128, 512], F32R, tag=f"wtqk{k}", name=f"wtqk{k}")
                nc.sync.dma_start(out=w[:], in_=wtqk_d[k * 128:(k + 1) * 128, :])
                wtqk_sb.append(w)
            wv_sb, pt_sb = [], []
            for k in range(2):
                w = wpool.tile([128, C], F32R, tag=f"wv{k}", name=f"wv{k}")
                nc.sync.dma_start(out=w[:], in_=wv_d[k * 128:(k + 1) * 128, :])
                wv_sb.append(w)
                p = wpool.tile([128, C], F32R, tag=f"pt{k}", name=f"pt{k}")
                nc.sync.dma_start(out=p[:], in_=pt_d[k * 128:(k + 1) * 128, :])
                pt_sb.append(p)
            gnw_sb, gnb_sb, bv_sb = [], [], []
            for k in range(2):
                sl = slice(k * 128, (k + 1) * 128)
                gw = small.tile([128, 1], F32, tag=f"gnw{k}", name=f"gnw{k}")
                nc.sync.dma_start(out=gw[:], in_=gnw_d[sl, :])
                gnw_sb.append(gw)
                gb = small.tile([128, 1], F32, tag=f"gnb{k}", name=f"gnb{k}")
                nc.sync.dma_start(out=gb[:], in_=gnb_d[sl, :])
                gnb_sb.append(gb)
                bv = small.tile([128, 1], F32R, tag=f"bv{k}", name=f"bv{k}")
                nc.sync.dma_start(out=bv[:], in_=bv_d[sl, :])
                bv_sb.append(bv)

            pb_sb = small.tile([1, C], F32, tag="pb", name="pb")
            nc.sync.dma_start(out=pb_sb[:], in_=pb_d[:])
            bqk_sb = small.tile([1, 512], F32R, tag="bqk", name="bqk")
            nc.sync.dma_start(out=bqk_sb[:], in_=bqk_d[:])
            g4_sb = small.tile([128, 4], F32, tag="g4", name="g4")
            nc.sync.dma_start(out=g4_sb[:], in_=g4_d[:])
            e4_sb = small.tile([4, 128], F32, tag="e4", name="e4")
            nc.sync.dma_start(out=e4_sb[:], in_=e4_d[:])
            dmask_sb = wpool.tile([128, 512], F32, tag="dmask", name="dmask")
            nc.sync.dma_start(out=dmask_sb[:], in_=dmask_d[:])

            eps41 = small.tile([4, 1], F32, tag="eps", name="eps")
            nc.gpsimd.memset(eps41[:], EPS)
            konst_sb = wpool.tile([128, 257], F32R, tag="konst", name="konst")
            nc.sync.dma_start(out=konst_sb[:], in_=const_d[:])
            one11 = konst_sb[0:1, 256:257]
            scr41 = small.tile([4, 1], F32, tag="scr", name="scr")
            # preload the sqrt activation table while DMAs run
            nc.scalar.activation(out=scr41[:], in_=eps41[:], func=ACT.Sqrt)

            def emit_front(reload_xt):
                """Gram blocks [G_b | S_b] -> cci, then the AllReduce."""
                if reload_xt:
                    for b in range(B):
                        nc.sync.dma_start(out=xt_sb[b][:], in_=xt_d[b])
                for b in range(B):
                    for ci in range(2):
                        t = b * 2 + ci
                        gps = p_g.tile([128, TW], F32, tag=f"g{b}{ci}", name=f"g{b}{ci}")
                        for k in range(NT):
                            nc.tensor.matmul(
                                gps[:],
                                xt_sb[b][:, k * TW + ci * 128: k * TW + ci * 128 + 128],
                                xt_sb[b][:, k * TW:(k + 1) * TW],
                                start=(k == 0), stop=(k == NT - 1),
                            )
                        gcp = small.tile([128, TW], F32R, tag=f"gcp{t}", name=f"gcp{t}")
                        if ci == 0:
                            nc.vector.tensor_copy(gcp[:], gps[:])
                        else:
                            nc.scalar.copy(out=gcp[:], in_=gps[:])
                        nc.sync.dma_start(out=cci[:, t * TW:(t + 1) * TW], in_=gcp[:])
                nc.gpsimd.collective_compute(
                    "AllReduce", ALU.add, replica_groups=rg, ins=[cci[:]], outs=[cco[:]]
                )

            def emit_back(reload_xs, upto=99):
                """post-AllReduce: stats chain, logits from G, softmax, fused
                per-batch weights, pass 2, output DMAs, next body's reloads."""
                gg = wpool.tile([128, CCW], F32R, tag="gg", name="gg")
                nc.sync.dma_start(out=gg[:], in_=cco[:])

                # ----- group stats from [G|S]: mean from S, E[x^2] from diag G -----
                st2x = small.tile([128, 8], F32, tag="st2x", name="st2x")
                for t in range(4):
                    nc.vector.tensor_copy(st2x[:, t:t + 1], gg[:, t * TW + C:t * TW + C + 1])
                for t in range(4):
                    ci = t % 2
                    dgt = small.tile([128, C], F32, tag=f"dgt{t % 2}", name=f"dgt{t}")
                    nc.vector.tensor_mul(
                        dgt[:], gg[:, t * TW:t * TW + C], dmask_sb[:, ci * C:(ci + 1) * C]
                    )
                    nc.vector.reduce_sum(
                        out=st2x[:, 4 + t:5 + t], in_=dgt[:], axis=mybir.AxisListType.X
                    )
                psum_g = p_misc.tile([4, 8], F32, tag="m", name="psum_g")
                nc.tensor.matmul(psum_g[:], g4_sb[:], st2x[:], start=True, stop=True)
                gsb = small.tile([4, 8], F32, tag="gsb", name="gsb")
                nc.vector.tensor_copy(gsb[:], psum_g[:])
                var44 = small.tile([4, 4], F32, tag="var44", name="var44")
                nc.vector.scalar_tensor_tensor(
                    out=var44[:], in0=gsb[:, 0:4], scalar=0.0, in1=gsb[:, 0:4],
                    op0=ALU.add, op1=ALU.mult,
                )  # mean^2
                nc.vector.tensor_sub(var44[:], gsb[:, 4:8], var44[:])
                rstd44 = small.tile([4, 4], F32, tag="rstd44", name="rstd44")
                nc.scalar.activation(
                    out=rstd44[:], in_=var44[:], func=ACT.Sqrt, bias=eps41[:], scale=1.0
                )
                nc.vector.reciprocal(out=rstd44[:], in_=rstd44[:])
                # preload the exp table right after the last sqrt
                nc.scalar.activation(out=scr41[:], in_=rstd44[:, 0:1], func=ACT.Exp)

                a_sb = [[None] * 2 for _ in range(B)]
                bb_sb = [[None] * 2 for _ in range(B)]
                wts_sb = [[None] * 2 for _ in range(B)]
                for b in range(B):
                    for cb in range(2):
                        t = b * 2 + cb
                        pmean = p_misc.tile([128, 1], F32, tag="m", name="pmean")
                        nc.tensor.matmul(
                            pmean[:], e4_sb[:], gsb[:, t:t + 1], start=True, stop=True
                        )
                        prstd = p_misc.tile([128, 1], F32, tag="m", name="prstd")
                        nc.tensor.matmul(
                            prstd[:], e4_sb[:], rstd44[:, t:t + 1], start=True, stop=True
                        )
                        a = small.tile([128, 1], F32, tag=f"a{t}", name=f"a{t}")
                        nc.vector.tensor_mul(a[:], prstd[:], gnw_sb[cb][:])
                        na = small.tile([128, 1], F32, tag=f"na{t}", name=f"na{t}")
                        nc.scalar.mul(out=na[:], in_=a[:], mul=-1.0)
                        bbv = small.tile([128, 1], F32R, tag=f"bb{t}", name=f"bb{t}")
                        nc.vector.scalar_tensor_tensor(
                            out=bbv[:], in0=pmean[:], scalar=na[:], in1=gnb_sb[cb][:],
                            op0=ALU.mult, op1=ALU.add,
                        )  # gnb - mean*a
                        w = wpool.tile([128, 512], F32R, tag=f"wts{t}", name=f"wts{t}")
                        nc.vector.tensor_scalar_mul(out=w[:], in0=wtqk_sb[cb][:], scalar1=a[:])
                        a_sb[b][cb], bb_sb[b][cb], wts_sb[b][cb] = a, bbv, w

                # rowbias rb (u|w), global colsums sg (AS|BS), rank-1 stacks
                lq_sb, rk_sb = [], []
                for b in range(B):
                    prb = p_misc.tile([1, 512], F32, tag="m", name="prb")
                    nc.tensor.matmul(prb[:], bb_sb[b][0][:], wtqk_sb[0][:], start=True, stop=False)
                    nc.tensor.matmul(prb[:], bb_sb[b][1][:], wtqk_sb[1][:], start=False, stop=False)
                    nc.tensor.matmul(prb[:], one11, bqk_sb[:], start=False, stop=True)
                    rb = small.tile([1, 512], F32, tag=f"rb{b}", name=f"rb{b}")
                    nc.vector.tensor_copy(rb[:], prb[:])
                    psg = p_misc.tile([1, 512], F32, tag="m", name="psg")
                    nc.tensor.matmul(psg[:], gg[:, (b * 2) * TW + C:(b * 2) * TW + C + 1],
                                     wts_sb[b][0][:], start=True, stop=False)
                    nc.tensor.matmul(psg[:], gg[:, (b * 2 + 1) * TW + C:(b * 2 + 1) * TW + C + 1],
                                     wts_sb[b][1][:], start=False, stop=True)
                    sg = small.tile([1, 512], F32, tag=f"sg{b}", name=f"sg{b}")
                    nc.vector.tensor_copy(sg[:], psg[:])
                    rbn = small.tile([1, 512], F32, tag=f"rbn{b}", name=f"rbn{b}")
                    nc.scalar.mul(out=rbn[:], in_=rb[:], mul=float(N))
                    lq = small.tile([3, 256], F32, tag=f"lq{b}", name=f"lq{b}")
                    nc.sync.dma_start(out=lq[0:1, :], in_=rb[0:1, 0:256])
                    nc.sync.dma_start(out=lq[1:2, :], in_=sg[0:1, 0:256])
                    nc.sync.dma_start(out=lq[2:3, :], in_=rbn[0:1, 0:256])
                    rk = small.tile([3, 256], F32, tag=f"rk{b}", name=f"rk{b}")
                    nc.sync.dma_start(out=rk[0:1, :], in_=sg[0:1, 256:512])
                    nc.sync.dma_start(out=rk[1:2, :], in_=rb[0:1, 256:512])
                    nc.sync.dma_start(out=rk[2:3, :], in_=rb[0:1, 256:512])
                    lq_sb.append(lq)
                    rk_sb.append(rk)

                if upto < 4:
                    return
                # ----- logits: M = G (diag(a) Wk^T) ; L = Wq_a^T M + rank-1 -----
                att_sm = [[None] * 2 for _ in range(B)]
                for b in range(B):
                    msb = wpool.tile([128, 2 * C], F32R, tag=f"msb{b}", name=f"msb{b}")
                    for ei in range(2):
                        mps = p_misc.tile([128, C], F32, tag="m", name="mps")
                        for ci in range(2):
                            t = b * 2 + ci
                            nc.tensor.matmul(
                                mps[:],
                                gg[:, t * TW + ei * 128: t * TW + ei * 128 + 128],
                                wts_sb[b][ci][:, 256:512],
                                start=(ci == 0), stop=(ci == 1),
                            )
                        nc.vector.tensor_copy(msb[:, ei * C:(ei + 1) * C], mps[:])
                    for ci in range(2):
                        lps = p_misc.tile([128, C], F32, tag="m", name="lps")
                        for ei in range(2):
                            nc.tensor.matmul(
                                lps[:],
                                wts_sb[b][ei][:, ci * 128: ci * 128 + 128],
                                msb[:, ei * C:(ei + 1) * C],
                                start=(ei == 0), stop=False,
                            )
                        nc.tensor.matmul(
                            lps[:],
                            lq_sb[b][:, ci * 128: ci * 128 + 128],
                            rk_sb[b][:],
                            start=False, stop=True, skip_group_check=True,
                        )
                        if upto < 5:
                            continue
                        # ----- extract head-diagonal blocks + softmax -----
                        atc = small.tile([128, 64], F32, tag=f"atc{ci}", name=f"atc{ci}")
                        nc.vector.tensor_copy(atc[0:64, :], lps[0:64, ci * 128: ci * 128 + 64])
                        nc.vector.tensor_copy(atc[64:128, :], lps[64:128, ci * 128 + 64: ci * 128 + 128])
                        negm = small.tile([128, 1], F32, tag=f"negm{ci}", name=f"negm{ci}")
                        nc.vector.reduce_max(
                            out=negm[:], in_=atc[:], axis=mybir.AxisListType.X, negate=True
                        )
                        nc.scalar.mul(out=negm[:], in_=negm[:], mul=SM_SCALE)
                        esb = small.tile([128, 64], F32, tag=f"esb{ci}", name=f"esb{ci}")
                        nc.scalar.activation(
                            out=esb[:], in_=atc[:], func=ACT.Exp,
                            bias=negm[:], scale=SM_SCALE,
                        )
                        ssum = small.tile([128, 1], F32, tag=f"ssum{ci}", name=f"ssum{ci}")
                        nc.vector.reduce_sum(out=ssum[:], in_=esb[:], axis=mybir.AxisListType.X)
                        nc.vector.reciprocal(out=ssum[:], in_=ssum[:])
                        sm = small.tile([128, 64], F32, tag=f"sm{b}{ci}", name=f"sm{b}{ci}")
                        nc.vector.tensor_scalar_mul(out=sm[:], in0=esb[:], scalar1=ssum[:])
                        att_sm[b][ci] = sm

                if upto < 6:
                    return
                # ----- blockdiag + fused per-batch weights -----
                gbt_sb = [[None] * 2 for _ in range(B)]
                mbt_sb = [[None] * 2 for _ in range(B)]
                beta_sb = [[None] * 2 for _ in range(B)]
                for b in range(B):
                    ablk = []
                    for k in range(2):
                        ab = wpool.tile([128, 256], F32R, tag=f"ablk{b}{k}", name=f"ablk{b}{k}")
                        nc.vector.tensor_copy(ab[:], konst_sb[:, 0:256])
                        h0, h1 = 2 * k, 2 * k + 1
                        nc.vector.tensor_copy(ab[0:64, h0 * 64:(h0 + 1) * 64], att_sm[b][k][0:64, :])
                        nc.vector.tensor_copy(ab[64:128, h1 * 64:(h1 + 1) * 64], att_sm[b][k][64:128, :])
                        ablk.append(ab)
                    for m in range(2):
                        pm = p_misc.tile([128, 256], F32, tag="m", name="pm")
                        msl = slice(m * 128, (m + 1) * 128)
                        nc.tensor.matmul(pm[:], ablk[0][:, msl], pt_sb[0][:], start=True, stop=False)
                        nc.tensor.matmul(pm[:], ablk[1][:, msl], pt_sb[1][:], start=False, stop=True)
                        mbt = wpool.tile([128, 256], F32R, tag=f"mbt{b}{m}", name=f"mbt{b}{m}")
                        nc.vector.tensor_copy(mbt[:], pm[:])
                        mbt_sb[b][m] = mbt
                    for g in range(2):
                        pg2 = p_misc.tile([128, 256], F32, tag="m", name="pg2")
                        gsl = slice(g * 128, (g + 1) * 128)
                        nc.tensor.matmul(pg2[:], wv_sb[0][:, gsl], mbt_sb[b][0][:], start=True, stop=False)
                        nc.tensor.matmul(pg2[:], wv_sb[1][:, gsl], mbt_sb[b][1][:], start=False, stop=True)
                        gbt = wpool.tile([128, 256], F32R, tag=f"gbt{b}{g}", name=f"gbt{b}{g}")
                        nc.vector.tensor_copy(gbt[:], pg2[:])
                        gbt_sb[b][g] = gbt
                    pbeta = p_misc.tile([1, C], F32, tag="m", name="pbeta")
                    nc.tensor.matmul(pbeta[:], bb_sb[b][0][:], gbt_sb[b][0][:], start=True, stop=False)
                    nc.tensor.matmul(pbeta[:], bb_sb[b][1][:], gbt_sb[b][1][:], start=False, stop=False)
                    nc.tensor.matmul(pbeta[:], bv_sb[0][:], mbt_sb[b][0][:], start=False, stop=False)
                    nc.tensor.matmul(pbeta[:], bv_sb[1][:], mbt_sb[b][1][:], start=False, stop=True)
                    brow = small.tile([1, C], F32, tag=f"brow{b}", name=f"brow{b}")
                    nc.vector.tensor_add(brow[:], pbeta[:], pb_sb[:])
                    for mo in range(2):
                        bet = small.tile([128, 1], F32, tag=f"beta{b}{mo}", name=f"beta{b}{mo}")
                        nc.sync.dma_start(out=bet[:], in_=brow[0:1, mo * 128:(mo + 1) * 128])
                        beta_sb[b][mo] = bet
                    # fold the GroupNorm scale into G_b (after the bias matmuls read it)
                    for g in range(2):
                        nc.vector.tensor_scalar_mul(
                            out=gbt_sb[b][g][:], in0=gbt_sb[b][g][:], scalar1=a_sb[b][g][:]
                        )

                if upto < 7:
                    return
                # ----- pass 2: out = G_b' x + beta + x, chunk-staged via SBUF -----
                for b in range(B):
                    for mo in range(2):
                        t = b * 2 + mo
                        msl = slice(mo * 128, (mo + 1) * 128)
                        for nt in range(Nc // 512):
                            nsl = slice(nt * 512, (nt + 1) * 512)
                            po = p_work.tile([128, 512], F32, tag="w", name="po")
                            nc.tensor.matmul(po[:], gbt_sb[b][0][:, msl], x_sb[b * 2][:, nsl],
                                             start=True, stop=False)
                            nc.tensor.matmul(po[:], gbt_sb[b][1][:, msl], x_sb[b * 2 + 1][:, nsl],
                                             start=False, stop=True)
                            osb = ochunk.tile([128, 512], F32, tag="o", name="osb")
                            nc.vector.scalar_tensor_tensor(
                                out=osb[:], in0=po[:], scalar=beta_sb[b][mo][:],
                                in1=x_sb[t][:, nsl], op0=ALU.add, op1=ALU.add,
                            )
                            nc.sync.dma_start(out=out_d[t][:, nsl], in_=osb[:])

                if reload_xs:
                    # next body's [c,n] copy + weights, after this body's reads
                    for t in range(4):
                        nc.sync.dma_start(out=x_sb[t][:], in_=xs_d[t])
                    for k in range(2):
                        nc.sync.dma_start(out=wtqk_sb[k][:], in_=wtqk_d[k * 128:(k + 1) * 128, :])
                        nc.sync.dma_start(out=wv_sb[k][:], in_=wv_d[k * 128:(k + 1) * 128, :])
                        nc.sync.dma_start(out=pt_sb[k][:], in_=pt_d[k * 128:(k + 1) * 128, :])

            if loop_r is None:
                R = unroll_r or 1
                emit_front(reload_xt=False)
                for r in range(1, R):
                    emit_front(reload_xt=True)
                    emit_back(reload_xs=True)
                emit_back(reload_xs=False)
            else:
                # timing variant: collective once, compute body looped
                emit_front(reload_xt=False)
                with tc.For_i(0, loop_r, 1):
                    for t in range(4):
                        nc.sync.dma_start(out=x_sb[t][:], in_=xs_d[t])
                    for b in range(B):
                        nc.sync.dma_start(out=xt_sb[b][:], in_=xt_d[b])
                    if upto >= 2:
                        for b in range(B):
                            for ci in range(2):
                                t = b * 2 + ci
                                gps = p_g.tile([128, TW], F32, tag=f"g{b}{ci}", name=f"lg{b}{ci}")
                                for k in range(NT):
                                    nc.tensor.matmul(
                                        gps[:],
                                        xt_sb[b][:, k * TW + ci * 128: k * TW + ci * 128 + 128],
                                        xt_sb[b][:, k * TW:(k + 1) * TW],
                                        start=(k == 0), stop=(k == NT - 1),
                                    )
                                nc.sync.dma_start(out=cci[:, t * TW:(t + 1) * TW], in_=gps[:])
                    if upto >= 3:
                        emit_back(reload_xs=False, upto=upto)

    if split_waits:
        _split_excess_waits(nc)
    return nc


_NC_CACHE = None


def _get_nc():
    global _NC_CACHE
    if _NC_CACHE is None:
        _NC_CACHE = build_nc()
    return _NC_CACHE


def _prep_inputs(x, gn_w, gn_b, qkv_w, qkv_b, proj_w, proj_b):
    x = np.ascontiguousarray(np.asarray(x, np.float32)).reshape(B, C, N)
    qkv_w = np.asarray(qkv_w, np.float32)
    qkv_b = np.asarray(qkv_b, np.float32)
    proj_w = np.asarray(proj_w, np.float32)
    shared = {
        "wtqk": np.ascontiguousarray(qkv_w[0:512].T),
        "wv": np.ascontiguousarray(qkv_w[512:768]),
        "pt": np.ascontiguousarray(proj_w.T),
        "gnw": np.asarray(gn_w, np.float32).reshape(C, 1),
        "gnb": np.asarray(gn_b, np.float32).reshape(C, 1),
        "bqk": qkv_b[0:512].reshape(1, 512),
        "bv": qkv_b[512:768].reshape(C, 1),
        "pb": np.asarray(proj_b, np.float32).reshape(1, C),
    }
    g4 = np.zeros((128, 4), np.float32)
    for p in range(128):
        g4[p, p // 32] = 1.0 / (32.0 * N)
    e4 = np.zeros((4, 128), np.float32)
    for p in range(128):
        e4[p // 32, p] = 1.0
    shared["g4"] = g4
    shared["e4"] = e4
    konst = np.zeros((128, 257), np.float32)
    konst[0, 256] = 1.0
    shared["konst"] = konst
    dmask = np.zeros((128, 512), np.float32)
    for p in range(128):
        dmask[p, p] = 1.0          # ci=0: diag at [p, p]
        dmask[p, 256 + 128 + p] = 1.0  # ci=1: diag at [p, 128+p]
    shared["dmask"] = dmask
    in_maps = []
    for s in range(S):
        xsh = x[:, :, s * Nc:(s + 1) * Nc]                      # [B, C, Nc]
        xs = np.ascontiguousarray(xsh).reshape(2 * B, 128, Nc)
        # [n,c] tiles + ones column: xt[b][p, k*TW + c] = xsh[b, c, k*128 + p]
        xt4 = xsh.transpose(0, 2, 1).reshape(B, NT, 128, C).transpose(0, 2, 1, 3)
        pad = np.zeros((B, 128, NT, 2), np.float32)
        pad[:, :, :, 0] = 1.0
        xt = np.concatenate([xt4, pad], axis=3).reshape(B, 128, NT * TW).astype(np.float32)
        in_maps.append({"xs": xs, "xt": np.ascontiguousarray(xt), **{k: v for k, v in shared.items()}})
    return in_maps


def kernel(x, gn_w, gn_b, qkv_w, qkv_b, proj_w, proj_b):
    nc = _get_nc()
    in_maps = _prep_inputs(x, gn_w, gn_b, qkv_w, qkv_b, proj_w, proj_b)
    res = run_bass_kernel_spmd(nc, in_maps, list(range(S)), trace=False)
    shards = [res.results[s]["out"].reshape(B, C, Nc) for s in range(S)]
    return np.concatenate(shards, axis=2).reshape(B, C, 32, 32, 32).astype(np.float32)
